# revision 9
# baseline (speedup 1.0000x reference)
"""DensityGridNN Trainium2 kernel (8 NeuronCores, SPMD).

Math (per batch b):
  grid = origin + linspace(0,1,48)*diag(lattice)        (separable in x,y,z)
  d2[g,n]  = (gx-px)^2 + (gy-py)^2 + (gz-pz)^2
  w[g,n]   = scale / (scale^2 + d2) * mask[n]
  c[g,:]   = w[g,:] @ nf                                 [G,128]  (output)
  out[g]   = relu(c[g,:] @ W1 + b1) @ W2 + b2            [G]      (output)

Sharding: core = (b, x-half). Each core computes 24 of the 48 x-planes of
one batch: 24*48*48 = 55296 grid points.

Host folds (exact, cancellation-free — all terms are sums of squares):
  dyz[n, iy*48+iz] = ((cy-py)^2 + (cz-pz)^2 + s^2)/s    [64, 2304]
  dx [n, ix]       = (cx-px)^2 / s                       per x-plane
so on device   v = dyz + dx   and   w = 1/v  = s/(s^2+d2).
Masking is folded into nf (zeroed atom rows), which is mathematically
identical (the mask only kills the contraction over atoms).
The decoder's first matmul is folded on host: NW = (nf*mask) @ W1, so
  hT = NW.T @ w   needs K=64 instead of K=128.

Device layout: everything transposed ([feature, grid] with grid in the free
dim). Two x-planes are packed into the 128 partitions (atoms 0-63 = plane A,
64-127 = plane B) so DVE/ACT run at full width; the matmuls use PE row-packing
(lhsT/rhs at base partitions 0 and 64) so the A/B matmuls run concurrently.
"""

import os
import sys

for _p in ("/opt/trn_rl_repo", "/root/.axon_site/_ro/trn_rl_repo"):
    if os.path.isdir(_p) and _p not in sys.path:
        sys.path.insert(0, _p)

import numpy as np

import concourse.bass as bass
import concourse.bacc as bacc
import concourse.mybir as mybir
from concourse.bass_utils import run_bass_kernel_spmd
from concourse.tile import TileContext

F32 = mybir.dt.float32

# ---- problem constants (hardcoded per the task contract) ----
GRID = 48
G_YZ = GRID * GRID          # 2304 grid points per x-plane
B, N, E = 4, 64, 128
H = 128                     # hidden width of the decoder
NX_HALF = 24                # x-planes per core
N_ITER = NX_HALF // 2       # 12: two x-planes per iteration
G_SLAB = NX_HALF * G_YZ     # 55296 grid points per core
N_CORES = 8

# ---- tunables ----
MM_DT = mybir.dt.float32r   # matmul compute dtype (float32r = 4x faster PE)
T_W = 768                   # free-dim slice per PSUM tile (2 banks)
T_SUB = [(0, 512), (512, 256)]   # matmul sub-slices (bank-aligned, N<=512)
OW = 384                    # out-vector matmul width (2 per 768 slice)
V_ENGINE = "vector"         # engine for v = dyz + dx
RELU_ACT = (0, 2, 4)        # which of the 6 relus per iter go to ScalarE
COPY_ACT = (0, 2, 4)        # which of the 6 xT copies per iter go to ScalarE

_CACHE = {}


def _build_nc():
    nc = bacc.Bacc()

    # consts packed into two tensors (one DMA each -> one wait each):
    # cst (f32):  [dyz2 | dxs | b1col]   cstr (f32r): [nf2 | nw2 | w2col]
    cst = nc.dram_tensor("cst", [128, G_YZ + N_ITER + 1], F32,
                         kind="ExternalInput")
    cstr = nc.dram_tensor("cstr", [128, E + H], MM_DT,
                          kind="ExternalInput")
    cstf = nc.dram_tensor("cstf", [128, 32], mybir.dt.float16,
                          kind="ExternalInput")

    cT = nc.dram_tensor("cT", [E, G_SLAB], F32, kind="ExternalOutput")
    ov = nc.dram_tensor("ov", [G_SLAB], F32, kind="ExternalOutput")

    with TileContext(nc) as tc:
        with (
            tc.tile_pool(name="const", bufs=1) as cpool,
            tc.tile_pool(name="work", bufs=2) as wpool,
            tc.tile_pool(name="stage", bufs=3) as spool,
            tc.tile_pool(name="ps", bufs=2, space="PSUM") as ppool,
        ):
            cst_t = cpool.tile_from(cst[:, :], force_copy=True)
            cstr_t = cpool.tile_from(cstr[:, :], force_copy=True)
            w2f_t = cpool.tile_from(cstf[:, :], force_copy=True)
            dyz_t = cst_t[:, 0:G_YZ]
            dxs_t = cst_t[:, G_YZ : G_YZ + N_ITER]
            b1_t = cst_t[:, G_YZ + N_ITER : G_YZ + N_ITER + 1]
            nf_t = cstr_t[:, 0:E]
            nw_t = cstr_t[:, E : E + H]

            for j in range(N_ITER):
                # v = dyz + dx_j   (= (s^2+d2)/s, strictly positive)
                v = wpool.tile([128, G_YZ], F32, tag="v")
                if V_ENGINE == "vector":
                    nc.vector.tensor_scalar_add(v, dyz_t, dxs_t[:, j : j + 1])
                elif V_ENGINE == "gpsimd":
                    nc.gpsimd.tensor_scalar_add(v, dyz_t, dxs_t[:, j : j + 1])
                else:
                    nc.scalar.activation(
                        v, dyz_t, mybir.ActivationFunctionType.Relu,
                        bias=dxs_t[:, j : j + 1],
                    )
                # w = 1/v  (Lorentzian weights, [atom, grid_yz] for 2 planes)
                w = wpool.tile([128, G_YZ], MM_DT, tag="w")
                with nc.allow_low_precision(reason="fp32r matmul operand"):
                    nc.vector.reciprocal(w, v)

                o_ps = None
                o_rows = []
                for s in range(2):          # the two x-planes of this iter
                    p0 = 64 * s
                    goff = (2 * j + s) * G_YZ
                    xcw = spool.tile([128, G_YZ], F32, tag="xc", bufs=2)
                    for t in range(3):      # 768-wide slices of the yz plane
                        off = T_W * t
                        six = s * 3 + t
                        # xT = nf.T @ w  -> [E, 768] slice of c (transposed)
                        xps = ppool.tile([128, T_W], F32, tag="xt", bufs=2)
                        for so, sw in T_SUB:
                            nc.tensor.matmul(
                                xps[:, so : so + sw],
                                lhsT=nf_t[p0 : p0 + 64, :],
                                rhs=w[p0 : p0 + 64, off + so : off + so + sw],
                                start=True, stop=True,
                            )
                        if six in COPY_ACT:
                            nc.scalar.copy(xcw[:, off : off + T_W], xps)
                        else:
                            nc.vector.tensor_copy(xcw[:, off : off + T_W], xps)
                        # hT = (nf@W1).T @ w -> pre-relu hidden, [H, 768]
                        hps = ppool.tile([128, T_W], F32, tag="ht", bufs=1)
                        for so, sw in T_SUB:
                            nc.tensor.matmul(
                                hps[:, so : so + sw],
                                lhsT=nw_t[p0 : p0 + 64, :],
                                rhs=w[p0 : p0 + 64, off + so : off + so + sw],
                                start=True, stop=True,
                            )
                        hr = spool.tile([128, T_W], mybir.dt.float16, tag="hr", bufs=3)
                        if six in RELU_ACT:
                            nc.scalar.activation(
                                hr, hps, mybir.ActivationFunctionType.Relu,
                                bias=b1_t[:, 0:1],
                            )
                        else:
                            nc.vector.tensor_scalar(
                                hr, hps, b1_t[:, 0:1], 0.0,
                                mybir.AluOpType.add, mybir.AluOpType.max,
                            )
                        # out = W2.T @ relu(hT): two 384-wide chunks,
                        # col-packed 4-to-a-PSUM-bank (partitions 0/32/64/96)
                        for k in range(2):
                            c_idx = six * 2 + k
                            row = 32 * (c_idx % 4)
                            if row == 0:
                                o_ps = ppool.tile([128, OW], F32, tag="o", bufs=2)
                                o_rows = []
                            nc.tensor.matmul(
                                o_ps[row : row + 32, :],
                                lhsT=w2f_t[:, :],
                                rhs=hr[:, k * OW : (k + 1) * OW],
                                start=True, stop=True,
                                tile_position=(0, row),
                            )
                            o_rows.append(goff + off + k * OW)
                            if len(o_rows) == 4:
                                # copy the whole bank (cost is free-size only;
                                # only partitions 0/32/64/96 hold data)
                                ost = spool.tile([128, OW], F32, tag="ost", bufs=3)
                                nc.vector.tensor_copy(ost, o_ps)
                                # DMA maximal contiguous runs (stepped
                                # partitions are fine for DMA)
                                r0 = 0
                                while r0 < 4:
                                    r1 = r0 + 1
                                    while (
                                        r1 < 4
                                        and o_rows[r1] == o_rows[r1 - 1] + OW
                                    ):
                                        r1 += 1
                                    nrun = r1 - r0
                                    dst = ov[
                                        o_rows[r0] : o_rows[r0] + nrun * OW
                                    ].rearrange("(p f) -> p f", p=nrun)
                                    nc.sync.dma_start(
                                        out=dst,
                                        in_=ost[32 * r0 : 32 * (r1 - 1) + 1 : 32, :],
                                    )
                                    r0 = r1
                    # write this plane's slice of c (transposed layout)
                    nc.sync.dma_start(
                        out=cT[:, goff : goff + G_YZ], in_=xcw
                    )
    nc.compile()
    return nc


def _prepare_in_maps(pos, node_features, origin, lattice, scale, W1, b1, W2,
                     b2, batch_nodes):
    scale = float(np.asarray(scale))
    diag = np.diagonal(np.asarray(lattice, np.float64), axis1=-2, axis2=-1)
    lin = np.linspace(0.0, 1.0, GRID)[None, None, :]
    # axes[b, xyz, i] = grid coordinate along each axis
    axes = np.asarray(origin, np.float64)[:, :, None] + lin * diag[:, :, None]
    posT = np.asarray(pos, np.float64).transpose(0, 2, 1)      # [B,3,N]
    d2 = (axes[:, :, None, :] - posT[:, :, :, None]) ** 2       # [B,3,N,48]
    s2 = scale * scale
    dyz = (d2[:, 1, :, :, None] + d2[:, 2, :, None, :] + s2) / scale
    dyzf = dyz.reshape(B, N, G_YZ).astype(np.float32)           # [B,N,2304]
    dxf = (d2[:, 0] / scale).astype(np.float32)                 # [B,N,48]

    mask = np.arange(N)[None, :] < np.asarray(batch_nodes)[:, None]
    nfm = np.asarray(node_features, np.float32) * mask[:, :, None].astype(
        np.float32
    )
    NW = (nfm.astype(np.float64) @ np.asarray(W1, np.float64)).astype(
        np.float32
    )
    w2col = np.ascontiguousarray(np.asarray(W2, np.float32).reshape(H, 1))
    b1col = np.ascontiguousarray(np.asarray(b1, np.float32).reshape(H, 1))

    in_maps = []
    for core in range(N_CORES):
        b, half = divmod(core, 2)
        ixs = half * NX_HALF + np.arange(NX_HALF)
        dxs = np.empty((128, N_ITER), np.float32)
        for jj in range(N_ITER):
            dxs[0:64, jj] = dxf[b, :, ixs[2 * jj]]
            dxs[64:128, jj] = dxf[b, :, ixs[2 * jj + 1]]
        dyz2 = np.concatenate([dyzf[b], dyzf[b]], axis=0)     # [128, 2304]
        nf2 = np.concatenate([nfm[b], nfm[b]], axis=0)        # [128, 128]
        nw2 = np.concatenate([NW[b], NW[b]], axis=0)          # [128, 128]
        b1f = np.broadcast_to(b1col, (128, 1))
        cst = np.ascontiguousarray(
            np.concatenate([dyz2, dxs, b1f], axis=1), np.float32
        )
        cstr = np.ascontiguousarray(
            np.concatenate([nf2, nw2], axis=1), np.float32
        )
        cstf = np.ascontiguousarray(
            np.repeat(w2col, 32, axis=1), np.float16
        )
        in_maps.append({"cst": cst, "cstr": cstr, "cstf": cstf})
    return in_maps


LAST_RESULTS = None


def kernel(pos, node_features, origin, lattice, scale, W1, b1, W2, b2,
           batch_nodes):
    global LAST_RESULTS
    if "nc" not in _CACHE:
        _CACHE["nc"] = _build_nc()
    nc = _CACHE["nc"]
    in_maps = _prepare_in_maps(
        pos, node_features, origin, lattice, scale, W1, b1, W2, b2,
        batch_nodes,
    )
    res = run_bass_kernel_spmd(nc, in_maps, core_ids=list(range(N_CORES)))
    LAST_RESULTS = res

    G = GRID * GRID * GRID
    c = np.empty((B, G, E), np.float32)
    out = np.empty((B, G), np.float32)
    b2f = float(np.asarray(b2).reshape(-1)[0])
    for core in range(N_CORES):
        b, half = divmod(core, 2)
        sl = slice(half * G_SLAB, (half + 1) * G_SLAB)
        c[b, sl, :] = res.results[core]["cT"].T
        out[b, sl] = res.results[core]["ov"] + b2f
    return (
        out.reshape(B, GRID, GRID, GRID),
        c.reshape(B, GRID, GRID, GRID, E),
    )


# revision 33
# speedup vs baseline: 1.9460x; 1.9460x over previous
"""DensityGridNN Trainium2 kernel (8 NeuronCores, SPMD).

Math (per batch b):
  grid = origin + linspace(0,1,48)*diag(lattice)        (separable in x,y,z)
  d2[g,n]  = (gx-px)^2 + (gy-py)^2 + (gz-pz)^2
  w[g,n]   = scale / (scale^2 + d2) * mask[n]
  c[g,:]   = w[g,:] @ nf                                 [G,128]  (output)
  out[g]   = relu(c[g,:] @ W1 + b1) @ W2 + b2            [G]      (output)

Sharding: core = (b, x-half). Each core computes 24 of the 48 x-planes of
one batch: 24*48*48 = 55296 grid points.

Host folds (exact, cancellation-free — all terms are sums of squares):
  dyz[n, iy*48+iz] = ((cy-py)^2 + (cz-pz)^2 + s^2)/s    [64, 2304]
  dx [n, ix]       = (cx-px)^2 / s                       per x-plane
so on device   v = dyz + dx   and   w = 1/v  = s/(s^2+d2).
Masking is folded into nf (zeroed atom rows), which is mathematically
identical (the mask only kills the contraction over atoms).
The decoder's first matmul is folded on host: NW = (nf*mask) @ W1, so
  hT = NW.T @ w   needs K=64 instead of K=128.

Device layout: everything transposed ([feature, grid] with grid in the free
dim). Two x-planes are packed into the 128 partitions (atoms 0-63 = plane A,
64-127 = plane B) so DVE/ACT run at full width; the matmuls use PE row-packing
(lhsT/rhs at base partitions 0 and 64) so the A/B matmuls run concurrently.
"""

import os
import sys

for _p in ("/opt/trn_rl_repo", "/root/.axon_site/_ro/trn_rl_repo"):
    if os.path.isdir(_p) and _p not in sys.path:
        sys.path.insert(0, _p)

import numpy as np

import concourse.bass as bass
import concourse.bacc as bacc
import concourse.mybir as mybir
from concourse.bass_utils import run_bass_kernel_spmd
from concourse.tile import TileContext

F32 = mybir.dt.float32

# ---- problem constants (hardcoded per the task contract) ----
GRID = 48
G_YZ = GRID * GRID          # 2304 grid points per x-plane
B, N, E = 4, 64, 128
H = 128                     # hidden width of the decoder
NX_HALF = 24                # x-planes per core
N_ITER = NX_HALF // 2       # 12: two x-planes per iteration
G_SLAB = NX_HALF * G_YZ     # 55296 grid points per core
N_CORES = 8

# ---- tunables ----
MM_DT = mybir.dt.float16    # matmul compute dtype
OW = 512                    # out-vector matmul width
V_ENGINE = "vector"         # engine for v = dyz + dx
RELU_ACT = (0, 3, 5, 8)     # chunk indices whose relu goes to ScalarE
COPY_ACT = tuple(range(10)) # chunk indices whose xT copy goes to ScalarE

_CACHE = {}


def _o_flush(nc, spool, ov, o_ps, o_dsts, use_act=False):
    """Evacuate a packed o PSUM bank (4 row-strips of up to 512 outputs).
    o_dsts[p] = (g_dst, width) for the strip at partitions 32p..32p+31.
    One engine copy, then DMA maximal contiguous runs via the (idle)
    GpSimd SWDGE queue to keep the SP sequencer free."""
    ost = spool.tile([128, 512], mybir.dt.float32, tag="ost", bufs=3)
    if use_act:
        nc.scalar.copy(ost, o_ps)
    else:
        nc.vector.tensor_copy(ost, o_ps)
    p = 0
    n = len(o_dsts)
    while p < n:
        if o_dsts[p] is None:
            p += 1
            continue
        d0, w0 = o_dsts[p]
        q = p + 1
        if w0 == 512:
            while q < n and o_dsts[q] is not None:
                dq, wq = o_dsts[q]
                if wq == 512 and dq == o_dsts[q - 1][0] + 512:
                    q += 1
                else:
                    break
        nrows = q - p
        dst = ov[d0 : d0 + nrows * w0].rearrange("(p f) -> p f", p=nrows)
        r0 = 32 * p
        src_ap = (
            ost[r0 : r0 + 32 * (nrows - 1) + 1 : 32, 0:w0]
            if nrows > 1
            else ost[r0 : r0 + 1, 0:w0]
        )
        nc.gpsimd.dma_start(out=dst, in_=src_ap)
        p = q


def _build_nc():
    nc = bacc.Bacc()

    # consts packed into two tensors (one DMA each -> one wait each):
    # cst (f32):  [dyz2 | dxs | b1col]   cstr (f32r): [nf2 | nw2 | w2col]
    cst = nc.dram_tensor("cst", [128, G_YZ + N_ITER + 1], F32,
                         kind="ExternalInput")
    cstr = nc.dram_tensor("cstr", [128, E + H], MM_DT,
                          kind="ExternalInput")
    cstf = nc.dram_tensor("cstf", [128, 32], mybir.dt.float16,
                          kind="ExternalInput")

    cT = nc.dram_tensor("cT", [E, G_SLAB], F32, kind="ExternalOutput")
    ov = nc.dram_tensor("ov", [G_SLAB], F32, kind="ExternalOutput")

    with TileContext(nc) as tc:
        with (
            tc.tile_pool(name="const", bufs=1) as cpool,
            tc.tile_pool(name="work", bufs=2) as wpool,
            tc.tile_pool(name="stage", bufs=3) as spool,
            tc.tile_pool(name="ps", bufs=2, space="PSUM") as ppool,
        ):
            cst_t = cpool.tile_from(cst[:, :], force_copy=True)
            cstr_t = cpool.tile_from(cstr[:, :], force_copy=True)
            w2f_t = cpool.tile_from(cstf[:, :], force_copy=True)
            dyz_t = cst_t[:, 0:G_YZ]
            dxs_t = cst_t[:, G_YZ : G_YZ + N_ITER]
            b1_t = cst_t[:, G_YZ + N_ITER : G_YZ + N_ITER + 1]
            nf_t = cstr_t[:, 0:E]
            nw_t = cstr_t[:, E : E + H]

            # HAM warmup: ~5us of dense fp32 matmuls (4 cyc/row) right after
            # the const DMA lands. PE_HAM unthrottles 1.2->2.4 GHz only after
            # a ~3.4us fully-busy window; the steady-state loop never has one
            # (mm bursts are evac-gated), but its idle gaps are all shorter
            # than the ~3.4us re-throttle window, so once warm it stays warm.
            wu_ps = ppool.tile([128, 512], F32, tag="wu", bufs=1)
            for _ in range(3):
                nc.tensor.matmul(
                    wu_ps[:, 0:512],
                    lhsT=cst_t[:, 0:128],
                    rhs=cst_t[:, 0:512],
                    start=True, stop=True,
                )

            for j in range(N_ITER):
                # v = dyz + dx_j   (= (s^2+d2)/s, strictly positive)
                v = wpool.tile([128, G_YZ], F32, tag="v", bufs=3)
                if V_ENGINE == "vector":
                    nc.vector.tensor_scalar_add(v, dyz_t, dxs_t[:, j : j + 1])
                elif V_ENGINE == "gpsimd":
                    nc.gpsimd.tensor_scalar_add(v, dyz_t, dxs_t[:, j : j + 1])
                else:
                    nc.scalar.activation(
                        v, dyz_t, mybir.ActivationFunctionType.Relu,
                        bias=dxs_t[:, j : j + 1],
                    )
                # w = 1/v  (Lorentzian weights, [atom, grid_yz] for 2 planes)
                # reciprocal_approx_fast (~51 ULP, ~5x faster than the native
                # RECIPROCAL which measures ~6 cyc/elem). Called via
                # _custom_dve so the output can be fp32r-typed (rounded on
                # store); the wrapper's f32-only assert is about the seed's
                # dependence on the *input* fp32 bit layout, which holds.
                from concourse.dve_ops import (
                    RECIP_APPROX_FAST_CONSTS,
                    RECIPROCAL_APPROX_FAST,
                )

                w = wpool.tile([128, G_YZ], MM_DT, tag="w", bufs=3)
                _c = RECIP_APPROX_FAST_CONSTS
                nc.vector._custom_dve(
                    RECIPROCAL_APPROX_FAST, out=w, in0=v,
                    s0=_c["s0"], s1=_c["s1"], imm2=_c["imm2"],
                )

                o_ps = None
                o_fill = 0
                o_dsts = []
                for s in range(2):          # the two x-planes of this iter
                    p0 = 64 * s
                    goff = (2 * j + s) * G_YZ
                    xcw = spool.tile([128, G_YZ], F32, tag="xc", bufs=3)
                    for t, (off, cw) in enumerate(
                        [(0, 512), (512, 512), (1024, 512), (1536, 512),
                         (2048, 256)]
                    ):
                        six = s * 5 + t
                        xps = ppool.tile([128, cw], F32, tag="xt", bufs=3,
                                         padded_shape=[128, 512])
                        nc.tensor.matmul(
                            xps[:, :],
                            lhsT=nf_t[p0 : p0 + 64, :],
                            rhs=w[p0 : p0 + 64, off : off + cw],
                            start=True, stop=True,
                        )
                        if six in COPY_ACT:
                            nc.scalar.copy(xcw[:, off : off + cw], xps)
                        else:
                            nc.vector.tensor_copy(xcw[:, off : off + cw], xps)
                        hps = ppool.tile([128, cw], F32, tag="ht", bufs=3,
                                         padded_shape=[128, 512])
                        nc.tensor.matmul(
                            hps[:, :],
                            lhsT=nw_t[p0 : p0 + 64, :],
                            rhs=w[p0 : p0 + 64, off : off + cw],
                            start=True, stop=True,
                        )
                        hr = spool.tile([128, cw], mybir.dt.float16, tag="hr",
                                        bufs=3, padded_shape=[128, 512])
                        if six in RELU_ACT:
                            nc.scalar.activation(
                                hr, hps, mybir.ActivationFunctionType.Relu,
                                bias=b1_t[:, 0:1],
                            )
                        else:
                            nc.vector.tensor_scalar(
                                hr, hps, b1_t[:, 0:1], 0.0,
                                mybir.AluOpType.add, mybir.AluOpType.max,
                            )
                        if o_ps is None:
                            o_ps = ppool.tile([128, OW], F32, tag="o",
                                              bufs=1)
                            o_fill = 0
                            o_dsts = []
                        row = 32 * o_fill
                        nc.tensor.matmul(
                            o_ps[row : row + 32, 0:cw],
                            lhsT=w2f_t[:, :],
                            rhs=hr[:, 0:cw],
                            start=True, stop=True,
                            tile_position=(0, row),
                        )
                        o_dsts.append((goff + off, cw))
                        o_fill += 1
                        if o_fill == 4:
                            _o_flush(nc, spool, ov, o_ps, o_dsts,
                                     use_act=(six % 2 == 0))
                            o_ps = None
                    nc.sync.dma_start(
                        out=cT[:, goff : goff + G_YZ], in_=xcw
                    )
                if o_ps is not None:
                    _o_flush(nc, spool, ov, o_ps, o_dsts)
                    o_ps = None
    nc.compile()
    return nc


def _prepare_in_maps(pos, node_features, origin, lattice, scale, W1, b1, W2,
                     b2, batch_nodes):
    scale = float(np.asarray(scale))
    diag = np.diagonal(np.asarray(lattice, np.float64), axis1=-2, axis2=-1)
    lin = np.linspace(0.0, 1.0, GRID)[None, None, :]
    # axes[b, xyz, i] = grid coordinate along each axis
    axes = np.asarray(origin, np.float64)[:, :, None] + lin * diag[:, :, None]
    posT = np.asarray(pos, np.float64).transpose(0, 2, 1)      # [B,3,N]
    d2 = (axes[:, :, None, :] - posT[:, :, :, None]) ** 2       # [B,3,N,48]
    s2 = scale * scale
    dyz = (d2[:, 1, :, :, None] + d2[:, 2, :, None, :] + s2) / scale
    dyzf = dyz.reshape(B, N, G_YZ).astype(np.float32)           # [B,N,2304]
    dxf = (d2[:, 0] / scale).astype(np.float32)                 # [B,N,48]

    mask = np.arange(N)[None, :] < np.asarray(batch_nodes)[:, None]
    nfm = np.asarray(node_features, np.float32) * mask[:, :, None].astype(
        np.float32
    )
    NW = (nfm.astype(np.float64) @ np.asarray(W1, np.float64)).astype(
        np.float32
    )
    w2col = np.ascontiguousarray(np.asarray(W2, np.float32).reshape(H, 1))
    b1col = np.ascontiguousarray(np.asarray(b1, np.float32).reshape(H, 1))

    in_maps = []
    for core in range(N_CORES):
        b, half = divmod(core, 2)
        ixs = half * NX_HALF + np.arange(NX_HALF)
        dxs = np.empty((128, N_ITER), np.float32)
        for jj in range(N_ITER):
            dxs[0:64, jj] = dxf[b, :, ixs[2 * jj]]
            dxs[64:128, jj] = dxf[b, :, ixs[2 * jj + 1]]
        dyz2 = np.concatenate([dyzf[b], dyzf[b]], axis=0)     # [128, 2304]
        nf2 = np.concatenate([nfm[b], nfm[b]], axis=0)        # [128, 128]
        nw2 = np.concatenate([NW[b], NW[b]], axis=0)          # [128, 128]
        b1f = np.broadcast_to(b1col, (128, 1))
        cst = np.ascontiguousarray(
            np.concatenate([dyz2, dxs, b1f], axis=1), np.float32
        )
        cstr = np.ascontiguousarray(
            np.concatenate([nf2, nw2], axis=1),
            np.float16 if MM_DT == mybir.dt.float16 else np.float32,
        )
        cstf = np.ascontiguousarray(
            np.repeat(w2col, 32, axis=1), np.float16
        )
        in_maps.append({"cst": cst, "cstr": cstr, "cstf": cstf})
    return in_maps


LAST_RESULTS = None


def kernel(pos, node_features, origin, lattice, scale, W1, b1, W2, b2,
           batch_nodes):
    global LAST_RESULTS
    if "nc" not in _CACHE:
        _CACHE["nc"] = _build_nc()
    nc = _CACHE["nc"]
    in_maps = _prepare_in_maps(
        pos, node_features, origin, lattice, scale, W1, b1, W2, b2,
        batch_nodes,
    )
    res = run_bass_kernel_spmd(nc, in_maps, core_ids=list(range(N_CORES)))
    LAST_RESULTS = res

    G = GRID * GRID * GRID
    c = np.empty((B, G, E), np.float32)
    out = np.empty((B, G), np.float32)
    b2f = float(np.asarray(b2).reshape(-1)[0])
    for core in range(N_CORES):
        b, half = divmod(core, 2)
        sl = slice(half * G_SLAB, (half + 1) * G_SLAB)
        c[b, sl, :] = res.results[core]["cT"].T
        out[b, sl] = res.results[core]["ov"] + b2f
    return (
        out.reshape(B, GRID, GRID, GRID),
        c.reshape(B, GRID, GRID, GRID, E),
    )


# revision 36
# speedup vs baseline: 2.0725x; 1.0650x over previous
"""DensityGridNN Trainium2 kernel (8 NeuronCores, SPMD).

Math (per batch b):
  grid = origin + linspace(0,1,48)*diag(lattice)        (separable in x,y,z)
  d2[g,n]  = (gx-px)^2 + (gy-py)^2 + (gz-pz)^2
  w[g,n]   = scale / (scale^2 + d2) * mask[n]
  c[g,:]   = w[g,:] @ nf                                 [G,128]  (output)
  out[g]   = relu(c[g,:] @ W1 + b1) @ W2 + b2            [G]      (output)

Sharding: core = (b, x-half). Each core computes 24 of the 48 x-planes of
one batch: 24*48*48 = 55296 grid points.

Host folds (exact, cancellation-free — all terms are sums of squares):
  dyz[n, iy*48+iz] = ((cy-py)^2 + (cz-pz)^2 + s^2)/s    [64, 2304]
  dx [n, ix]       = (cx-px)^2 / s                       per x-plane
so on device   v = dyz + dx   and   w = 1/v  = s/(s^2+d2).
Masking is folded into nf (zeroed atom rows), which is mathematically
identical (the mask only kills the contraction over atoms).
The decoder's first matmul is folded on host: NW = (nf*mask) @ W1, so
  hT = NW.T @ w   needs K=64 instead of K=128.

Device layout: everything transposed ([feature, grid] with grid in the free
dim). Two x-planes are packed into the 128 partitions (atoms 0-63 = plane A,
64-127 = plane B) so DVE/ACT run at full width; the matmuls use PE row-packing
(lhsT/rhs at base partitions 0 and 64) so the A/B matmuls run concurrently.
"""

import os
import sys

for _p in ("/opt/trn_rl_repo", "/root/.axon_site/_ro/trn_rl_repo"):
    if os.path.isdir(_p) and _p not in sys.path:
        sys.path.insert(0, _p)

import numpy as np

import concourse.bacc as bacc
import concourse.mybir as mybir
from concourse.bass_utils import run_bass_kernel_spmd
from concourse.tile import TileContext

F32 = mybir.dt.float32

# ---- problem constants (hardcoded per the task contract) ----
GRID = 48
G_YZ = GRID * GRID          # 2304 grid points per x-plane
B, N, E = 4, 64, 128
H = 128                     # hidden width of the decoder
NX_HALF = 24                # x-planes per core
N_ITER = NX_HALF // 2       # 12: two x-planes per iteration
G_SLAB = NX_HALF * G_YZ     # 55296 grid points per core
N_CORES = 8

# ---- tunables ----
MM_DT = mybir.dt.float16    # matmul compute dtype
OW = 512                    # out-vector matmul width
V_ENGINE = "vector"         # engine for v = dyz + dx
RELU_ACT = (0, 3, 5, 8)     # chunk indices whose relu goes to ScalarE
COPY_ACT = tuple(range(10)) # chunk indices whose xT copy goes to ScalarE

_CACHE = {}


def _o_flush(nc, spool, ov, o_ps, o_dsts, use_act=False):
    """Evacuate a packed o PSUM bank (4 row-strips of up to 512 outputs).
    o_dsts[p] = (g_dst, width) for the strip at partitions 32p..32p+31.
    One engine copy, then DMA maximal contiguous runs via the (idle)
    GpSimd SWDGE queue to keep the SP sequencer free."""
    ost = spool.tile([128, 512], mybir.dt.float32, tag="ost", bufs=4)
    if use_act:
        nc.scalar.copy(ost, o_ps)
    else:
        nc.vector.tensor_copy(ost, o_ps)
    p = 0
    n = len(o_dsts)
    while p < n:
        if o_dsts[p] is None:
            p += 1
            continue
        d0, w0 = o_dsts[p]
        q = p + 1
        if w0 == 512:
            while q < n and o_dsts[q] is not None:
                dq, wq = o_dsts[q]
                if wq == 512 and dq == o_dsts[q - 1][0] + 512:
                    q += 1
                else:
                    break
        nrows = q - p
        dst = ov[d0 : d0 + nrows * w0].rearrange("(p f) -> p f", p=nrows)
        r0 = 32 * p
        src_ap = (
            ost[r0 : r0 + 32 * (nrows - 1) + 1 : 32, 0:w0]
            if nrows > 1
            else ost[r0 : r0 + 1, 0:w0]
        )
        nc.gpsimd.dma_start(out=dst, in_=src_ap)
        p = q


def _build_nc():
    nc = bacc.Bacc()

    # consts packed into three tensors (one DMA each -> one wait each):
    # cst (f32): [dyz2 | dxs | b1col]; cstr (fp16): [nf2 | nw2];
    # cstf (fp16): W2 replicated x32 (col-group-packed out matmuls)
    cst = nc.dram_tensor("cst", [128, G_YZ + N_ITER + 1], F32,
                         kind="ExternalInput")
    cstr = nc.dram_tensor("cstr", [128, E + H], MM_DT,
                          kind="ExternalInput")
    cstf = nc.dram_tensor("cstf", [128, 32], mybir.dt.float16,
                          kind="ExternalInput")

    cT = nc.dram_tensor("cT", [E, G_SLAB], F32, kind="ExternalOutput")
    ov = nc.dram_tensor("ov", [G_SLAB], F32, kind="ExternalOutput")

    with TileContext(nc) as tc:
        with (
            tc.tile_pool(name="const", bufs=1) as cpool,
            tc.tile_pool(name="work", bufs=2) as wpool,
            tc.tile_pool(name="stage", bufs=3) as spool,
            tc.tile_pool(name="ps", bufs=2, space="PSUM") as ppool,
        ):
            cst_t = cpool.tile_from(cst[:, :], force_copy=True)
            cstr_t = cpool.tile_from(cstr[:, :], force_copy=True)
            w2f_t = cpool.tile_from(cstf[:, :], force_copy=True)
            dyz_t = cst_t[:, 0:G_YZ]
            dxs_t = cst_t[:, G_YZ : G_YZ + N_ITER]
            b1_t = cst_t[:, G_YZ + N_ITER : G_YZ + N_ITER + 1]
            nf_t = cstr_t[:, 0:E]
            nw_t = cstr_t[:, E : E + H]

            for j in range(N_ITER):
                # v = dyz + dx_j   (= (s^2+d2)/s, strictly positive)
                v = wpool.tile([128, G_YZ], F32, tag="v", bufs=3)
                if V_ENGINE == "vector":
                    nc.vector.tensor_scalar_add(v, dyz_t, dxs_t[:, j : j + 1])
                elif V_ENGINE == "gpsimd":
                    nc.gpsimd.tensor_scalar_add(v, dyz_t, dxs_t[:, j : j + 1])
                else:
                    nc.scalar.activation(
                        v, dyz_t, mybir.ActivationFunctionType.Relu,
                        bias=dxs_t[:, j : j + 1],
                    )
                # w = 1/v  (Lorentzian weights, [atom, grid_yz] for 2 planes)
                # reciprocal_approx_fast (~51 ULP, ~5x faster than the native
                # RECIPROCAL which measures ~6 cyc/elem). Called via
                # _custom_dve so the output can be fp32r-typed (rounded on
                # store); the wrapper's f32-only assert is about the seed's
                # dependence on the *input* fp32 bit layout, which holds.
                from concourse.dve_ops import (
                    RECIP_APPROX_FAST_CONSTS,
                    RECIPROCAL_APPROX_FAST,
                )

                w = wpool.tile([128, G_YZ], MM_DT, tag="w", bufs=3)
                _c = RECIP_APPROX_FAST_CONSTS
                nc.vector._custom_dve(
                    RECIPROCAL_APPROX_FAST, out=w, in0=v,
                    s0=_c["s0"], s1=_c["s1"], imm2=_c["imm2"],
                )

                o_ps = None
                o_fill = 0
                o_dsts = []
                for s in range(2):          # the two x-planes of this iter
                    p0 = 64 * s
                    goff = (2 * j + s) * G_YZ
                    xcw = spool.tile([128, G_YZ], F32, tag="xc", bufs=4)
                    for t, (off, cw) in enumerate(
                        [(0, 512), (512, 512), (1024, 512), (1536, 512),
                         (2048, 256)]
                    ):
                        six = s * 5 + t
                        xps = ppool.tile([128, cw], F32, tag="xt", bufs=3,
                                         padded_shape=[128, 512])
                        nc.tensor.matmul(
                            xps[:, :],
                            lhsT=nf_t[p0 : p0 + 64, :],
                            rhs=w[p0 : p0 + 64, off : off + cw],
                            start=True, stop=True,
                        )
                        if six in COPY_ACT:
                            nc.scalar.copy(xcw[:, off : off + cw], xps)
                        else:
                            nc.vector.tensor_copy(xcw[:, off : off + cw], xps)
                        hps = ppool.tile([128, cw], F32, tag="ht", bufs=3,
                                         padded_shape=[128, 512])
                        nc.tensor.matmul(
                            hps[:, :],
                            lhsT=nw_t[p0 : p0 + 64, :],
                            rhs=w[p0 : p0 + 64, off : off + cw],
                            start=True, stop=True,
                        )
                        hr = spool.tile([128, cw], mybir.dt.float16, tag="hr",
                                        bufs=4, padded_shape=[128, 512])
                        if six in RELU_ACT:
                            nc.scalar.activation(
                                hr, hps, mybir.ActivationFunctionType.Relu,
                                bias=b1_t[:, 0:1],
                            )
                        else:
                            nc.vector.tensor_scalar(
                                hr, hps, b1_t[:, 0:1], 0.0,
                                mybir.AluOpType.add, mybir.AluOpType.max,
                            )
                        if o_ps is None:
                            o_ps = ppool.tile([128, OW], F32, tag="o",
                                              bufs=2)
                            o_fill = 0
                            o_dsts = []
                        row = 32 * o_fill
                        nc.tensor.matmul(
                            o_ps[row : row + 32, 0:cw],
                            lhsT=w2f_t[:, :],
                            rhs=hr[:, 0:cw],
                            start=True, stop=True,
                            tile_position=(0, row),
                        )
                        o_dsts.append((goff + off, cw))
                        o_fill += 1
                        if o_fill == 4:
                            _o_flush(nc, spool, ov, o_ps, o_dsts,
                                     use_act=(six % 2 == 0))
                            o_ps = None
                    nc.sync.dma_start(
                        out=cT[:, goff : goff + G_YZ], in_=xcw
                    )
                if o_ps is not None:
                    _o_flush(nc, spool, ov, o_ps, o_dsts)
                    o_ps = None
    nc.compile()
    return nc


def _prepare_in_maps(pos, node_features, origin, lattice, scale, W1, b1, W2,
                     b2, batch_nodes):
    scale = float(np.asarray(scale))
    diag = np.diagonal(np.asarray(lattice, np.float64), axis1=-2, axis2=-1)
    lin = np.linspace(0.0, 1.0, GRID)[None, None, :]
    # axes[b, xyz, i] = grid coordinate along each axis
    axes = np.asarray(origin, np.float64)[:, :, None] + lin * diag[:, :, None]
    posT = np.asarray(pos, np.float64).transpose(0, 2, 1)      # [B,3,N]
    d2 = (axes[:, :, None, :] - posT[:, :, :, None]) ** 2       # [B,3,N,48]
    s2 = scale * scale
    dyz = (d2[:, 1, :, :, None] + d2[:, 2, :, None, :] + s2) / scale
    dyzf = dyz.reshape(B, N, G_YZ).astype(np.float32)           # [B,N,2304]
    dxf = (d2[:, 0] / scale).astype(np.float32)                 # [B,N,48]

    mask = np.arange(N)[None, :] < np.asarray(batch_nodes)[:, None]
    nfm = np.asarray(node_features, np.float32) * mask[:, :, None].astype(
        np.float32
    )
    NW = (nfm.astype(np.float64) @ np.asarray(W1, np.float64)).astype(
        np.float32
    )
    w2col = np.ascontiguousarray(np.asarray(W2, np.float32).reshape(H, 1))
    b1col = np.ascontiguousarray(np.asarray(b1, np.float32).reshape(H, 1))

    in_maps = []
    for core in range(N_CORES):
        b, half = divmod(core, 2)
        ixs = half * NX_HALF + np.arange(NX_HALF)
        dxs = np.empty((128, N_ITER), np.float32)
        for jj in range(N_ITER):
            dxs[0:64, jj] = dxf[b, :, ixs[2 * jj]]
            dxs[64:128, jj] = dxf[b, :, ixs[2 * jj + 1]]
        dyz2 = np.concatenate([dyzf[b], dyzf[b]], axis=0)     # [128, 2304]
        nf2 = np.concatenate([nfm[b], nfm[b]], axis=0)        # [128, 128]
        nw2 = np.concatenate([NW[b], NW[b]], axis=0)          # [128, 128]
        b1f = np.broadcast_to(b1col, (128, 1))
        cst = np.ascontiguousarray(
            np.concatenate([dyz2, dxs, b1f], axis=1), np.float32
        )
        cstr = np.ascontiguousarray(
            np.concatenate([nf2, nw2], axis=1),
            np.float16 if MM_DT == mybir.dt.float16 else np.float32,
        )
        cstf = np.ascontiguousarray(
            np.repeat(w2col, 32, axis=1), np.float16
        )
        in_maps.append({"cst": cst, "cstr": cstr, "cstf": cstf})
    return in_maps


LAST_RESULTS = None


def kernel(pos, node_features, origin, lattice, scale, W1, b1, W2, b2,
           batch_nodes):
    global LAST_RESULTS
    if "nc" not in _CACHE:
        _CACHE["nc"] = _build_nc()
    nc = _CACHE["nc"]
    in_maps = _prepare_in_maps(
        pos, node_features, origin, lattice, scale, W1, b1, W2, b2,
        batch_nodes,
    )
    res = run_bass_kernel_spmd(nc, in_maps, core_ids=list(range(N_CORES)))
    LAST_RESULTS = res

    G = GRID * GRID * GRID
    c = np.empty((B, G, E), np.float32)
    out = np.empty((B, G), np.float32)
    b2f = float(np.asarray(b2).reshape(-1)[0])
    for core in range(N_CORES):
        b, half = divmod(core, 2)
        sl = slice(half * G_SLAB, (half + 1) * G_SLAB)
        c[b, sl, :] = res.results[core]["cT"].T
        out[b, sl] = res.results[core]["ov"] + b2f
    return (
        out.reshape(B, GRID, GRID, GRID),
        c.reshape(B, GRID, GRID, GRID, E),
    )


# revision 38
# speedup vs baseline: 2.0738x; 1.0006x over previous
"""DensityGridNN Trainium2 kernel (8 NeuronCores, SPMD).

Math (per batch b):
  grid = origin + linspace(0,1,48)*diag(lattice)        (separable in x,y,z)
  d2[g,n]  = (gx-px)^2 + (gy-py)^2 + (gz-pz)^2
  w[g,n]   = scale / (scale^2 + d2) * mask[n]
  c[g,:]   = w[g,:] @ nf                                 [G,128]  (output)
  out[g]   = relu(c[g,:] @ W1 + b1) @ W2 + b2            [G]      (output)

Sharding: core = (b, x-half). Each core computes 24 of the 48 x-planes of
one batch: 24*48*48 = 55296 grid points.

Host folds (exact, cancellation-free — all terms are sums of squares):
  dyz[n, iy*48+iz] = ((cy-py)^2 + (cz-pz)^2 + s^2)/s    [64, 2304]
  dx [n, ix]       = (cx-px)^2 / s                       per x-plane
so on device   v = dyz + dx   and   w = 1/v  = s/(s^2+d2).
Masking is folded into nf (zeroed atom rows), which is mathematically
identical (the mask only kills the contraction over atoms).
The decoder's first matmul is folded on host: NW = (nf*mask) @ W1, so
  hT = NW.T @ w   needs K=64 instead of K=128.

Device layout: everything transposed ([feature, grid] with grid in the free
dim). Two x-planes are packed into the 128 partitions (atoms 0-63 = plane A,
64-127 = plane B) so DVE/ACT run at full width; the matmuls use PE row-packing
(lhsT/rhs at base partitions 0 and 64) so the A/B matmuls run concurrently.
"""

import os
import sys

for _p in ("/opt/trn_rl_repo", "/root/.axon_site/_ro/trn_rl_repo"):
    if os.path.isdir(_p) and _p not in sys.path:
        sys.path.insert(0, _p)

import numpy as np

import concourse.bacc as bacc
import concourse.mybir as mybir
from concourse.bass_utils import run_bass_kernel_spmd
from concourse.tile import TileContext

F32 = mybir.dt.float32

# ---- problem constants (hardcoded per the task contract) ----
GRID = 48
G_YZ = GRID * GRID          # 2304 grid points per x-plane
B, N, E = 4, 64, 128
H = 128                     # hidden width of the decoder
NX_HALF = 24                # x-planes per core
N_ITER = NX_HALF // 2       # 12: two x-planes per iteration
G_SLAB = NX_HALF * G_YZ     # 55296 grid points per core
N_CORES = 8

# ---- tunables ----
MM_DT = mybir.dt.float16    # matmul compute dtype
OW = 512                    # out-vector matmul width
V_ENGINE = "vector"         # engine for v = dyz + dx
RELU_ACT = (0, 3, 5, 8)     # chunk indices whose relu goes to ScalarE
COPY_ACT = tuple(range(10)) # chunk indices whose xT copy goes to ScalarE

_CACHE = {}


def _o_flush(nc, spool, ov, o_ps, o_dsts, use_act=False):
    """Evacuate a packed o PSUM bank (4 row-strips of up to 512 outputs).
    o_dsts[p] = (g_dst, width) for the strip at partitions 32p..32p+31.
    One engine copy, then DMA maximal contiguous runs via the (idle)
    GpSimd SWDGE queue to keep the SP sequencer free."""
    ost = spool.tile([128, 512], mybir.dt.float32, tag="ost", bufs=4)
    if use_act:
        nc.scalar.copy(ost, o_ps)
    else:
        nc.vector.tensor_copy(ost, o_ps)
    p = 0
    n = len(o_dsts)
    while p < n:
        if o_dsts[p] is None:
            p += 1
            continue
        d0, w0 = o_dsts[p]
        q = p + 1
        if w0 == 512:
            while q < n and o_dsts[q] is not None:
                dq, wq = o_dsts[q]
                if wq == 512 and dq == o_dsts[q - 1][0] + 512:
                    q += 1
                else:
                    break
        nrows = q - p
        dst = ov[d0 : d0 + nrows * w0].rearrange("(p f) -> p f", p=nrows)
        r0 = 32 * p
        src_ap = (
            ost[r0 : r0 + 32 * (nrows - 1) + 1 : 32, 0:w0]
            if nrows > 1
            else ost[r0 : r0 + 1, 0:w0]
        )
        nc.gpsimd.dma_start(out=dst, in_=src_ap)
        p = q


def _build_nc():
    nc = bacc.Bacc()

    # consts packed into three tensors (one DMA each -> one wait each):
    # cst (f32): [dyz2 | dxs | b1col]; cstr (fp16): [nf2 | nw2];
    # cstf (fp16): W2 replicated x32 (col-group-packed out matmuls)
    cst = nc.dram_tensor("cst", [128, G_YZ + N_ITER + 1], F32,
                         kind="ExternalInput")
    cstr = nc.dram_tensor("cstr", [128, E + H], MM_DT,
                          kind="ExternalInput")
    cstf = nc.dram_tensor("cstf", [128, 32], mybir.dt.float16,
                          kind="ExternalInput")

    cT = nc.dram_tensor("cT", [E, G_SLAB], F32, kind="ExternalOutput")
    ov = nc.dram_tensor("ov", [G_SLAB], F32, kind="ExternalOutput")

    with TileContext(nc) as tc:
        with (
            tc.tile_pool(name="const", bufs=1) as cpool,
            tc.tile_pool(name="work", bufs=2) as wpool,
            tc.tile_pool(name="stage", bufs=3) as spool,
            tc.tile_pool(name="ps", bufs=2, space="PSUM") as ppool,
        ):
            cst_t = cpool.tile_from(cst[:, :], force_copy=True)
            cstr_t = cpool.tile_from(cstr[:, :], force_copy=True)
            w2f_t = cpool.tile_from(cstf[:, :], force_copy=True)
            dyz_t = cst_t[:, 0:G_YZ]
            dxs_t = cst_t[:, G_YZ : G_YZ + N_ITER]
            b1_t = cst_t[:, G_YZ + N_ITER : G_YZ + N_ITER + 1]
            nf_t = cstr_t[:, 0:E]
            nw_t = cstr_t[:, E : E + H]

            for j in range(N_ITER):
                # v = dyz + dx_j   (= (s^2+d2)/s, strictly positive)
                v = wpool.tile([128, G_YZ], F32, tag="v", bufs=3)
                if V_ENGINE == "vector":
                    nc.vector.tensor_scalar_add(v, dyz_t, dxs_t[:, j : j + 1])
                elif V_ENGINE == "gpsimd":
                    nc.gpsimd.tensor_scalar_add(v, dyz_t, dxs_t[:, j : j + 1])
                else:
                    nc.scalar.activation(
                        v, dyz_t, mybir.ActivationFunctionType.Relu,
                        bias=dxs_t[:, j : j + 1],
                    )
                # w = 1/v  (Lorentzian weights, [atom, grid_yz] for 2 planes)
                # reciprocal_approx_fast (~51 ULP, ~5x faster than the native
                # RECIPROCAL which measures ~6 cyc/elem). Called via
                # _custom_dve so the output can be fp32r-typed (rounded on
                # store); the wrapper's f32-only assert is about the seed's
                # dependence on the *input* fp32 bit layout, which holds.
                from concourse.dve_ops import (
                    RECIP_APPROX_FAST_CONSTS,
                    RECIPROCAL_APPROX_FAST,
                )

                w = wpool.tile([128, G_YZ], MM_DT, tag="w", bufs=3)
                _c = RECIP_APPROX_FAST_CONSTS
                nc.vector._custom_dve(
                    RECIPROCAL_APPROX_FAST, out=w, in0=v,
                    s0=_c["s0"], s1=_c["s1"], imm2=_c["imm2"],
                )

                o_ps = None
                o_fill = 0
                o_dsts = []
                for s in range(2):          # the two x-planes of this iter
                    p0 = 64 * s
                    goff = (2 * j + s) * G_YZ
                    xcw = spool.tile([128, G_YZ], F32, tag="xc", bufs=4)
                    for t, (off, cw) in enumerate(
                        [(0, 512), (512, 512), (1024, 512), (1536, 512),
                         (2048, 256)]
                    ):
                        six = s * 5 + t
                        xps = ppool.tile([128, cw], F32, tag="xt", bufs=3,
                                         padded_shape=[128, 512])
                        nc.tensor.matmul(
                            xps[:, :],
                            lhsT=nf_t[p0 : p0 + 64, :],
                            rhs=w[p0 : p0 + 64, off : off + cw],
                            start=True, stop=True,
                        )
                        if six in COPY_ACT:
                            nc.scalar.copy(xcw[:, off : off + cw], xps)
                        else:
                            nc.vector.tensor_copy(xcw[:, off : off + cw], xps)
                        hps = ppool.tile([128, cw], F32, tag="ht", bufs=3,
                                         padded_shape=[128, 512])
                        nc.tensor.matmul(
                            hps[:, :],
                            lhsT=nw_t[p0 : p0 + 64, :],
                            rhs=w[p0 : p0 + 64, off : off + cw],
                            start=True, stop=True,
                        )
                        hr = spool.tile([128, cw], mybir.dt.float16, tag="hr",
                                        bufs=4, padded_shape=[128, 512])
                        if six in RELU_ACT:
                            nc.scalar.activation(
                                hr, hps, mybir.ActivationFunctionType.Relu,
                                bias=b1_t[:, 0:1],
                            )
                        else:
                            nc.vector.tensor_scalar(
                                hr, hps, b1_t[:, 0:1], 0.0,
                                mybir.AluOpType.add, mybir.AluOpType.max,
                            )
                        if o_ps is None:
                            o_ps = ppool.tile([128, OW], F32, tag="o",
                                              bufs=2)
                            o_fill = 0
                            o_dsts = []
                        row = 32 * o_fill
                        nc.tensor.matmul(
                            o_ps[row : row + 32, 0:cw],
                            lhsT=w2f_t[:, :],
                            rhs=hr[:, 0:cw],
                            start=True, stop=True,
                            tile_position=(0, row),
                        )
                        o_dsts.append((goff + off, cw))
                        o_fill += 1
                        if o_fill == 4:
                            _o_flush(nc, spool, ov, o_ps, o_dsts,
                                     use_act=(six % 2 == 0))
                            o_ps = None
                    nc.sync.dma_start(
                        out=cT[:, goff : goff + G_YZ], in_=xcw
                    )
                if o_ps is not None:
                    _o_flush(nc, spool, ov, o_ps, o_dsts)
                    o_ps = None
    nc.compile()
    return nc


def _prepare_in_maps(pos, node_features, origin, lattice, scale, W1, b1, W2,
                     b2, batch_nodes):
    scale = float(np.asarray(scale))
    diag = np.diagonal(np.asarray(lattice, np.float64), axis1=-2, axis2=-1)
    lin = np.linspace(0.0, 1.0, GRID)[None, None, :]
    # axes[b, xyz, i] = grid coordinate along each axis
    axes = np.asarray(origin, np.float64)[:, :, None] + lin * diag[:, :, None]
    posT = np.asarray(pos, np.float64).transpose(0, 2, 1)      # [B,3,N]
    d2 = (axes[:, :, None, :] - posT[:, :, :, None]) ** 2       # [B,3,N,48]
    s2 = scale * scale
    dyz = (d2[:, 1, :, :, None] + d2[:, 2, :, None, :] + s2) / scale
    dyzf = dyz.reshape(B, N, G_YZ).astype(np.float32)           # [B,N,2304]
    dxf = (d2[:, 0] / scale).astype(np.float32)                 # [B,N,48]

    mask = np.arange(N)[None, :] < np.asarray(batch_nodes)[:, None]
    nfm = np.asarray(node_features, np.float32) * mask[:, :, None].astype(
        np.float32
    )
    NW = (nfm.astype(np.float64) @ np.asarray(W1, np.float64)).astype(
        np.float32
    )
    w2col = np.ascontiguousarray(np.asarray(W2, np.float32).reshape(H, 1))
    b1col = np.ascontiguousarray(np.asarray(b1, np.float32).reshape(H, 1))

    in_maps = []
    for core in range(N_CORES):
        b, half = divmod(core, 2)
        ixs = half * NX_HALF + np.arange(NX_HALF)
        dxs = np.empty((128, N_ITER), np.float32)
        for jj in range(N_ITER):
            dxs[0:64, jj] = dxf[b, :, ixs[2 * jj]]
            dxs[64:128, jj] = dxf[b, :, ixs[2 * jj + 1]]
        dyz2 = np.concatenate([dyzf[b], dyzf[b]], axis=0)     # [128, 2304]
        nf2 = np.concatenate([nfm[b], nfm[b]], axis=0)        # [128, 128]
        nw2 = np.concatenate([NW[b], NW[b]], axis=0)          # [128, 128]
        b1f = np.broadcast_to(b1col, (128, 1))
        cst = np.ascontiguousarray(
            np.concatenate([dyz2, dxs, b1f], axis=1), np.float32
        )
        cstr = np.ascontiguousarray(
            np.concatenate([nf2, nw2], axis=1),
            np.float16 if MM_DT == mybir.dt.float16 else np.float32,
        )
        cstf = np.ascontiguousarray(
            np.repeat(w2col, 32, axis=1), np.float16
        )
        in_maps.append({"cst": cst, "cstr": cstr, "cstf": cstf})
    return in_maps


LAST_RESULTS = None


def kernel(pos, node_features, origin, lattice, scale, W1, b1, W2, b2,
           batch_nodes):
    global LAST_RESULTS
    if "nc" not in _CACHE:
        _CACHE["nc"] = _build_nc()
    nc = _CACHE["nc"]
    in_maps = _prepare_in_maps(
        pos, node_features, origin, lattice, scale, W1, b1, W2, b2,
        batch_nodes,
    )
    res = run_bass_kernel_spmd(nc, in_maps, core_ids=list(range(N_CORES)))
    LAST_RESULTS = res

    G = GRID * GRID * GRID
    c = np.empty((B, G, E), np.float32)
    out = np.empty((B, G), np.float32)
    b2f = float(np.asarray(b2).reshape(-1)[0])
    for core in range(N_CORES):
        b, half = divmod(core, 2)
        sl = slice(half * G_SLAB, (half + 1) * G_SLAB)
        c[b, sl, :] = res.results[core]["cT"].T
        out[b, sl] = res.results[core]["ov"] + b2f
    return (
        out.reshape(B, GRID, GRID, GRID),
        c.reshape(B, GRID, GRID, GRID, E),
    )
